# revision 11
# baseline (speedup 1.0000x reference)
"""Trainium2 Bass kernel for nn_Aggregator (GNN relational message passing).

Computes: out[h] = sum_{e: head_e==h} all_emb[tail_e] * weight[type_e] * aug_e

Strategy (8 NeuronCores, SPMD):
  - Shard output nodes (head ranges of 12500) across the 8 cores; each core
    gets exactly the edges whose head falls in its range (host bins them).
    No collective needed - host concatenates the 8 output shards.
  - Host sorts edges by head into 64-node output tiles (196 per core).
    Within a tile, edges are grouped by tail window (4 windows of 25000
    source rows; dma_gather indices are int16) and each (tile, window) run
    is padded to a fixed capacity (capw 128-slot chunks) so the device
    schedule is fully static. Within each bucket edges are sorted by tail
    for HBM locality during the gather.
  - Gather: one dma_gather per (14-tile group, window) pulls all_emb rows
    (256B each) into SBUF, window-major == processing order (slot order).
  - Per (group, window) slab of 28 chunks (one chunk = 128 edges):
      Wsel = ohT^T @ W4stack on PE (fp16)  -> aug_e*weight[type_e]  [e,64]
      S    = is_equal(head, iota64) on DVE -> head one-hot (fp16)   [e,64]
      V    = G * Wsel on DVE (fp16 out)                             [e,64]
      acc[tile] (psum f32, 64 partitions) += S_q^T @ V_q per chunk
    Group flush: 14 PSUM->SBUF staging copies on ScalarE (2 tiles packed
    per 128 partitions); one output DMA at the end.
"""

import os

import numpy as np

import concourse.bacc as bacc
import concourse.tile as tile
from concourse import bass, mybir
from concourse.bass_utils import run_bass_kernel_spmd

P = 128
C = 64  # channels
R = 24  # relations
RP = 32  # relations padded (4 chunks of 32 rows stack into 128 partitions)
TS = 64  # output-tile size in nodes
N_NODES = 100000
N_CORES = 8
NPC = N_NODES // N_CORES  # 12500 nodes per core
TILES = (NPC + TS - 1) // TS  # 196 output tiles per core
WINS = 4
WSZ = 25000  # window size (int16 gather index reach)
GROUP_T = 14  # tiles per gather group (196 = 14 * 14)
NGRP = TILES // GROUP_T

_NC_CACHE = {}


def _build(capw: int):
    """Build the Bass module for per-(tile,window) chunk capacity capw."""
    cap = WINS * capw             # chunks per tile
    nchunk = TILES * cap          # chunks per core
    cpw = GROUP_T * capw          # chunks per (group, window)
    gc = WINS * cpw               # chunks per group
    ncol = (nchunk // 4) * P      # onehot columns

    nc = bacc.Bacc("TRN2", target_bir_lowering=False, num_swdge_queues=4)

    f32 = mybir.dt.float32
    f16 = mybir.dt.float16
    i16 = mybir.dt.int16

    emb_d = nc.dram_tensor("all_emb", [N_NODES, C], f32, kind="ExternalInput")
    idx_d = nc.dram_tensor("idx16", [P, nchunk * 8], i16, kind="ExternalInput")
    head_d = nc.dram_tensor("head_local", [P, nchunk], f16,
                            kind="ExternalInput")
    oh_d = nc.dram_tensor("oh", [P, ncol], f16, kind="ExternalInput")
    w4_d = nc.dram_tensor("w4", [P, 4 * C], f16, kind="ExternalInput")
    iota_d = nc.dram_tensor("iota", [P, TS], f16, kind="ExternalInput")
    out_d = nc.dram_tensor("out", [TILES * TS, C], f32, kind="ExternalOutput")

    with tile.TileContext(nc) as tc:
        with (
            tc.tile_pool(name="res", bufs=1) as res,
            tc.tile_pool(name="gp", bufs=4) as gp,
            tc.tile_pool(name="ixp", bufs=3) as ixp,
            tc.tile_pool(name="ohp", bufs=2) as ohp,
            tc.tile_pool(name="sp", bufs=6) as sp,
            tc.tile_pool(name="vp", bufs=6) as vp,
            tc.tile_pool(name="osp", bufs=3) as osp,
            tc.tile_pool(name="wselp", bufs=1, space="PSUM") as wselp,
            tc.tile_pool(name="accp", bufs=4, space="PSUM") as accp,
        ):
            head_t = res.tile([P, nchunk], f16)
            iota_t = res.tile([P, TS], f16)
            w4_t = res.tile([P, 4 * C], f16)

            nc.sync.dma_start(head_t[:], head_d[:])
            nc.sync.dma_start(iota_t[:], iota_d[:])
            nc.sync.dma_start(w4_t[:], w4_d[:])

            for g in range(NGRP):
                # stream this group's gather indices
                idx_t = ixp.tile([P, gc * 8], i16, tag="idx")
                nc.sync.dma_start(
                    idx_t[:], idx_d[:, g * gc * 8 : (g + 1) * gc * 8]
                )

                # gather the whole group, one call per source window;
                # slot order == processing order (window-major, then tile)
                g_t = gp.tile([P, gc * C], f32, tag="g")
                for w in range(WINS):
                    nidx = cpw * P
                    nc.gpsimd.dma_gather(
                        out_ap=g_t[:, w * cpw * C : (w + 1) * cpw * C]
                        .rearrange("p (j c) -> p j c", c=C),
                        in_ap=emb_d[w * WSZ : min((w + 1) * WSZ, N_NODES), :],
                        idxs_ap=idx_t[:, w * cpw * 8 : w * cpw * 8 + nidx // 16],
                        num_idxs=nidx,
                        num_idxs_reg=nidx,
                        elem_size=C,
                        single_packet=False,
                        queue_num=w,
                    )

                # onehot slab for the group's chunks
                oh_t = ohp.tile([P, (gc // 4) * P], f16, tag="oh")
                nc.sync.dma_start(
                    oh_t[:], oh_d[:, (g * gc // 4) * P : ((g + 1) * gc // 4) * P]
                )

                # materialize S/V slabs for all 4 windows of the group
                s_w = []
                v_w = []
                for w in range(WINS):
                    j0 = g * gc + w * cpw  # first chunk of the slab

                    # head one-hots for the slab's chunks (fp16)
                    s_t = sp.tile([P, cpw * TS], f16, tag="s")
                    nc.vector.tensor_tensor(
                        out=s_t[:].rearrange("p (q n) -> p q n", n=TS),
                        in0=head_t[:, j0 : j0 + cpw].unsqueeze(2).to_broadcast(
                            [P, cpw, TS]
                        ),
                        in1=iota_t[:].unsqueeze(1).to_broadcast([P, cpw, TS]),
                        op=mybir.AluOpType.is_equal,
                    )
                    s_w.append(s_t)

                    # Wsel for the slab: one stacked matmul per 4 chunks
                    wsel_t = wselp.tile([P, cpw * C], f32, tag="wsel")
                    for b in range(cpw // 4):
                        gcol = (w * cpw // 4 + b) * P
                        nc.tensor.matmul(
                            out=wsel_t[:, 4 * b * C : 4 * (b + 1) * C],
                            lhsT=oh_t[:, gcol : gcol + P],
                            rhs=w4_t[:],
                            start=True,
                            stop=True,
                        )

                    # V = G * Wsel for the whole slab in one DVE op
                    v_t = vp.tile([P, cpw * C], f16, tag="v")
                    nc.vector.tensor_tensor(
                        out=v_t[:],
                        in0=g_t[:, w * cpw * C : (w + 1) * cpw * C],
                        in1=wsel_t[:],
                        op=mybir.AluOpType.mult,
                    )
                    v_w.append(v_t)

                # scatter: per tile-pair PSUM bank; within a pair the two
                # tiles' accumulation groups run strictly one after another
                # (even tile -> partitions 0-63, odd -> 64-127 via col
                # tile_position), so only one group is ever open per bank
                ostage = osp.tile([P, (GROUP_T // 2) * C], f32, tag="ost")
                for m in range(GROUP_T // 2):
                    acc_t = accp.tile([P, 512], f32, tag="acc")
                    for tb in range(2):
                        tloc = 2 * m + tb
                        pb = TS * tb
                        nmm = WINS * capw
                        for i in range(nmm):
                            w = i // capw
                            kw = i % capw
                            q = tloc * capw + kw
                            nc.tensor.matmul(
                                out=acc_t[pb : pb + TS, :C],
                                lhsT=s_w[w][:, q * TS : (q + 1) * TS],
                                rhs=v_w[w][:, q * C : (q + 1) * C],
                                start=(i == 0),
                                stop=(i == nmm - 1),
                            )
                    nc.scalar.copy(
                        out=ostage[:, m * C : (m + 1) * C],
                        in_=acc_t[:, :C],
                    )

                # write this group's 14 output tiles (overlaps later groups)
                nc.sync.dma_start(
                    out_d[g * GROUP_T * TS : (g + 1) * GROUP_T * TS]
                    .rearrange("(t2 tb p) c -> (tb p) t2 c", tb=2, p=TS),
                    ostage[:].rearrange("p (t2 c) -> p t2 c", c=C),
                )

    nc.finalize()
    return nc


def _prep(all_emb, edge_index, edge_type, weight, aug_edge_weight):
    """Host-side binning/padding. Returns (capw, in_maps)."""
    head = np.asarray(edge_index[0], dtype=np.int64)
    tail = np.asarray(edge_index[1], dtype=np.int64)
    etype = np.asarray(edge_type, dtype=np.int64)
    aug = np.asarray(aug_edge_weight, dtype=np.float32).reshape(-1)
    emb = np.ascontiguousarray(np.asarray(all_emb, dtype=np.float32))
    w = np.asarray(weight, dtype=np.float32)

    order = np.argsort(head, kind="stable")
    h_s = head[order]
    bounds = np.searchsorted(h_s, np.arange(N_CORES + 1) * NPC)

    capw = 1
    per_core = []
    for c_i in range(N_CORES):
        e_idx = order[bounds[c_i] : bounds[c_i + 1]]
        h_loc = h_s[bounds[c_i] : bounds[c_i + 1]] - c_i * NPC
        t_loc = tail[e_idx]
        tw = (h_loc // TS) * WINS + t_loc // WSZ  # (tile, window) bucket
        cnt = np.bincount(tw, minlength=TILES * WINS)
        capw = max(capw, int(-(-cnt.max() // P)))
        per_core.append((e_idx, h_loc, t_loc, tw, cnt))

    cap = WINS * capw
    nchunk = TILES * cap
    cpw = GROUP_T * capw
    gc = WINS * cpw
    ncol = (nchunk // 4) * P

    iota = np.tile(np.arange(TS, dtype=np.float16), (P, 1))
    w4 = np.zeros((P, 4 * C), dtype=np.float16)
    for s in range(4):
        w4[RP * s : RP * s + R, s * C : (s + 1) * C] = w

    in_maps = []
    for c_i in range(N_CORES):
        e_idx, h_loc, t_loc, tw, cnt = per_core[c_i]
        # group by (tile, window); sort by tail within each bucket (HBM
        # locality for the gather stream)
        o2 = np.lexsort((t_loc, tw))
        e_idx, h_loc, t_loc, tw = e_idx[o2], h_loc[o2], t_loc[o2], tw[o2]
        starts = np.cumsum(cnt) - cnt
        pos = np.arange(len(e_idx)) - starts[tw]

        tile_id = tw // WINS
        win = tw % WINS
        kw = pos >> 7
        p = pos & (P - 1)

        grp = tile_id // GROUP_T
        tloc = tile_id % GROUP_T
        # chunk index == gather slot (window-major within group, then tile)
        j = grp * gc + win * cpw + tloc * capw + kw
        gi = j * P + p  # flat gather slot

        idx16 = np.zeros((P, nchunk * 8), np.int16)
        val16 = (t_loc - win * WSZ).astype(np.int16)
        rows = (gi % 16).astype(np.int64)
        cols = (gi // 16).astype(np.int64)
        for rep in range(8):
            idx16[rep * 16 + rows, cols] = val16

        head_a = np.zeros((P, nchunk), dtype=np.float16)
        head_a[p, j] = (h_loc - tile_id * TS).astype(np.float16)

        oh = np.zeros((P, ncol), dtype=np.float16)
        q_r = RP * (j & 3) + etype[e_idx]
        col = (j >> 2) * P + p
        oh[q_r, col] = aug[e_idx]

        in_maps.append(
            {
                "all_emb": emb,
                "idx16": idx16,
                "head_local": head_a,
                "oh": oh,
                "w4": w4,
                "iota": iota,
            }
        )
    return capw, in_maps


def kernel(all_emb, edge_index, edge_type, weight, aug_edge_weight):
    capw, in_maps = _prep(all_emb, edge_index, edge_type, weight,
                          aug_edge_weight)
    if capw not in _NC_CACHE:
        _NC_CACHE[capw] = _build(capw)
    nc = _NC_CACHE[capw]

    trace = bool(int(os.environ.get("KERNEL_TRACE", "0")))
    res = run_bass_kernel_spmd(
        nc,
        in_maps,
        core_ids=list(range(N_CORES)),
        trace=trace,
    )
    kernel._last_results = res
    out = np.concatenate(
        [res.results[c_i]["out"][:NPC] for c_i in range(N_CORES)], axis=0
    )
    return out


# revision 14
# speedup vs baseline: 1.1266x; 1.1266x over previous
"""Trainium2 Bass kernel for nn_Aggregator (GNN relational message passing).

Computes: out[h] = sum_{e: head_e==h} all_emb[tail_e] * weight[type_e] * aug_e

Strategy (8 NeuronCores, SPMD):
  - Shard output nodes (head ranges of 12500) across the 8 cores; each core
    gets exactly the edges whose head falls in its range (host bins them).
    No collective needed - host concatenates the 8 output shards.
  - Host sorts edges by head into 64-node output tiles (196 per core).
    Within a tile, edges are grouped by tail window (4 windows of 25000
    source rows; dma_gather indices are int16) and each (tile, window) run
    is padded to a fixed capacity (capw 128-slot chunks) so the device
    schedule is fully static. Within each bucket edges are sorted by tail
    for HBM locality during the gather.
  - Gather: one dma_gather per (14-tile group, window) pulls all_emb rows
    (256B each) into SBUF, window-major == processing order (slot order).
  - Per (group, window) slab of 28 chunks (one chunk = 128 edges):
      Wsel = ohT^T @ W4stack on PE (fp16)  -> aug_e*weight[type_e]  [e,64]
      S    = is_equal(head, iota64) on DVE -> head one-hot (fp16)   [e,64]
      V    = G * Wsel on DVE (fp16 out)                             [e,64]
      acc[tile] (psum f32, 64 partitions) += S_q^T @ V_q per chunk
    Group flush: 14 PSUM->SBUF staging copies on ScalarE (2 tiles packed
    per 128 partitions); one output DMA at the end.
"""

import os

import numpy as np

import concourse.bacc as bacc
import concourse.tile as tile
from concourse import bass, mybir
from concourse.bass_utils import run_bass_kernel_spmd

P = 128
C = 64  # channels
R = 24  # relations
RP = 32  # relations padded (4 chunks of 32 rows stack into 128 partitions)
TS = 64  # output-tile size in nodes
N_NODES = 100000
N_CORES = 8
NPC = N_NODES // N_CORES  # 12500 nodes per core
TILES = (NPC + TS - 1) // TS  # 196 output tiles per core
WINS = 4
WSZ = 25000  # window size (int16 gather index reach)
def _group_t(capw: int) -> int:
    # tiles per gather group; shrink groups for larger capacities so the
    # gather buffers / PSUM wsel slab still fit (196 = 14*14 = 28*7)
    return 14 if capw <= 2 else 7

_NC_CACHE = {}


def _build(capw: int):
    """Build the Bass module for per-(tile,window) chunk capacity capw."""
    GROUP_T = _group_t(capw)
    NGRP = TILES // GROUP_T
    cap = WINS * capw             # chunks per tile
    nchunk = TILES * cap          # chunks per core
    cpw = GROUP_T * capw          # chunks per (group, window)
    gc = WINS * cpw               # chunks per group
    ncol = (nchunk // 4) * P      # onehot columns

    nc = bacc.Bacc("TRN2", target_bir_lowering=False, num_swdge_queues=4)

    f32 = mybir.dt.float32
    f16 = mybir.dt.float16
    i16 = mybir.dt.int16

    emb_d = nc.dram_tensor("all_emb", [N_NODES, C], f32, kind="ExternalInput")
    idx_d = nc.dram_tensor("idx16", [P, nchunk * 8], i16, kind="ExternalInput")
    head_d = nc.dram_tensor("head_local", [P, nchunk], f16,
                            kind="ExternalInput")
    oh_d = nc.dram_tensor("oh", [P, ncol], f16, kind="ExternalInput")
    w4_d = nc.dram_tensor("w4", [P, 4 * C], f16, kind="ExternalInput")
    iota_d = nc.dram_tensor("iota", [P, TS], f16, kind="ExternalInput")
    out_d = nc.dram_tensor("out", [TILES * TS, C], f32, kind="ExternalOutput")

    with tile.TileContext(nc) as tc:
        with (
            tc.tile_pool(name="res", bufs=1) as res,
            tc.tile_pool(name="gp", bufs=3) as gp,
            tc.tile_pool(name="ohp", bufs=2) as ohp,
            tc.tile_pool(name="sp", bufs=6) as sp,
            tc.tile_pool(name="vp", bufs=6) as vp,
            tc.tile_pool(name="wselp", bufs=1, space="PSUM") as wselp,
            tc.tile_pool(name="accp", bufs=4, space="PSUM") as accp,
        ):
            idx_t = res.tile([P, nchunk * 8], i16)
            head_t = res.tile([P, nchunk], f16)
            iota_t = res.tile([P, TS], f16)
            w4_t = res.tile([P, 4 * C], f16)
            ostage = res.tile([P, (TILES // 2) * C], f32)

            nc.sync.dma_start(idx_t[:], idx_d[:])
            nc.sync.dma_start(head_t[:], head_d[:])
            nc.sync.dma_start(iota_t[:], iota_d[:])
            nc.sync.dma_start(w4_t[:], w4_d[:])

            for g in range(NGRP):
                # gather the whole group, one call per source window;
                # slot order == processing order (window-major, then tile)
                g_t = gp.tile([P, gc * C], f32, tag="g")
                for w in range(WINS):
                    nidx = cpw * P
                    s0 = g * gc + w * cpw  # first slot of this call
                    nc.gpsimd.dma_gather(
                        out_ap=g_t[:, w * cpw * C : (w + 1) * cpw * C]
                        .rearrange("p (j c) -> p j c", c=C),
                        in_ap=emb_d[w * WSZ : min((w + 1) * WSZ, N_NODES), :],
                        idxs_ap=idx_t[:, s0 * 8 : s0 * 8 + nidx // 16],
                        num_idxs=nidx,
                        num_idxs_reg=nidx,
                        elem_size=C,
                        single_packet=False,
                        queue_num=w,
                    )

                # onehot slab for the group's chunks
                oh_t = ohp.tile([P, (gc // 4) * P], f16, tag="oh")
                nc.sync.dma_start(
                    oh_t[:], oh_d[:, (g * gc // 4) * P : ((g + 1) * gc // 4) * P]
                )

                # materialize S/V slabs for all 4 windows of the group
                s_w = []
                v_w = []
                for w in range(WINS):
                    j0 = g * gc + w * cpw  # first chunk of the slab

                    # head one-hots for the slab's chunks (fp16)
                    s_t = sp.tile([P, cpw * TS], f16, tag="s")
                    nc.vector.tensor_tensor(
                        out=s_t[:].rearrange("p (q n) -> p q n", n=TS),
                        in0=head_t[:, j0 : j0 + cpw].unsqueeze(2).to_broadcast(
                            [P, cpw, TS]
                        ),
                        in1=iota_t[:].unsqueeze(1).to_broadcast([P, cpw, TS]),
                        op=mybir.AluOpType.is_equal,
                    )
                    s_w.append(s_t)

                    # Wsel for the slab: one stacked matmul per 4 chunks
                    wsel_t = wselp.tile([P, cpw * C], f32, tag="wsel")
                    for b in range(cpw // 4):
                        gcol = (w * cpw // 4 + b) * P
                        nc.tensor.matmul(
                            out=wsel_t[:, 4 * b * C : 4 * (b + 1) * C],
                            lhsT=oh_t[:, gcol : gcol + P],
                            rhs=w4_t[:],
                            start=True,
                            stop=True,
                        )

                    # V = G * Wsel for the whole slab in one DVE op
                    v_t = vp.tile([P, cpw * C], f16, tag="v")
                    nc.vector.tensor_tensor(
                        out=v_t[:],
                        in0=g_t[:, w * cpw * C : (w + 1) * cpw * C],
                        in1=wsel_t[:],
                        op=mybir.AluOpType.mult,
                    )
                    v_w.append(v_t)

                # scatter: per tile-pair PSUM bank; within a pair the two
                # tiles' accumulation groups run strictly one after another
                # (even tile -> partitions 0-63, odd -> 64-127 via col
                # tile_position), so only one group is ever open per bank
                for m in range(GROUP_T // 2):
                    acc_t = accp.tile([P, 512], f32, tag="acc")
                    for tb in range(2):
                        tloc = 2 * m + tb
                        pb = TS * tb
                        nmm = WINS * capw
                        for i in range(nmm):
                            w = i // capw
                            kw = i % capw
                            q = tloc * capw + kw
                            nc.tensor.matmul(
                                out=acc_t[pb : pb + TS, :C],
                                lhsT=s_w[w][:, q * TS : (q + 1) * TS],
                                rhs=v_w[w][:, q * C : (q + 1) * C],
                                start=(i == 0),
                                stop=(i == nmm - 1),
                            )
                    t2 = g * (GROUP_T // 2) + m
                    nc.scalar.copy(
                        out=ostage[:, t2 * C : (t2 + 1) * C],
                        in_=acc_t[:, :C],
                    )

            nc.sync.dma_start(
                out_d[:].rearrange("(t2 tb p) c -> (tb p) t2 c", tb=2, p=TS),
                ostage[:].rearrange("p (t2 c) -> p t2 c", c=C),
            )

    nc.finalize()
    return nc


def _prep(all_emb, edge_index, edge_type, weight, aug_edge_weight):
    """Host-side binning/padding. Returns (capw, in_maps)."""
    head = np.asarray(edge_index[0], dtype=np.int64)
    tail = np.asarray(edge_index[1], dtype=np.int64)
    etype = np.asarray(edge_type, dtype=np.int64)
    aug = np.asarray(aug_edge_weight, dtype=np.float32).reshape(-1)
    emb = np.ascontiguousarray(np.asarray(all_emb, dtype=np.float32))
    w = np.asarray(weight, dtype=np.float32)

    order = np.argsort(head, kind="stable")
    h_s = head[order]
    bounds = np.searchsorted(h_s, np.arange(N_CORES + 1) * NPC)

    capw = 1
    per_core = []
    for c_i in range(N_CORES):
        e_idx = order[bounds[c_i] : bounds[c_i + 1]]
        h_loc = h_s[bounds[c_i] : bounds[c_i + 1]] - c_i * NPC
        t_loc = tail[e_idx]
        tw = (h_loc // TS) * WINS + t_loc // WSZ  # (tile, window) bucket
        cnt = np.bincount(tw, minlength=TILES * WINS)
        capw = max(capw, int(-(-cnt.max() // P)))
        per_core.append((e_idx, h_loc, t_loc, tw, cnt))

    GROUP_T = _group_t(capw)
    cap = WINS * capw
    nchunk = TILES * cap
    cpw = GROUP_T * capw
    gc = WINS * cpw
    ncol = (nchunk // 4) * P

    iota = np.tile(np.arange(TS, dtype=np.float16), (P, 1))
    w4 = np.zeros((P, 4 * C), dtype=np.float16)
    for s in range(4):
        w4[RP * s : RP * s + R, s * C : (s + 1) * C] = w

    in_maps = []
    for c_i in range(N_CORES):
        e_idx, h_loc, t_loc, tw, cnt = per_core[c_i]
        # group by (tile, window); sort by tail within each bucket (HBM
        # locality for the gather stream)
        o2 = np.lexsort((t_loc, tw))
        e_idx, h_loc, t_loc, tw = e_idx[o2], h_loc[o2], t_loc[o2], tw[o2]
        starts = np.cumsum(cnt) - cnt
        pos = np.arange(len(e_idx)) - starts[tw]

        tile_id = tw // WINS
        win = tw % WINS
        kw = pos >> 7
        p = pos & (P - 1)

        grp = tile_id // GROUP_T
        tloc = tile_id % GROUP_T
        # chunk index == gather slot (window-major within group, then tile)
        j = grp * gc + win * cpw + tloc * capw + kw
        gi = j * P + p  # flat gather slot

        idx16 = np.zeros((P, nchunk * 8), np.int16)
        val16 = (t_loc - win * WSZ).astype(np.int16)
        rows = (gi % 16).astype(np.int64)
        cols = (gi // 16).astype(np.int64)
        for rep in range(8):
            idx16[rep * 16 + rows, cols] = val16

        head_a = np.zeros((P, nchunk), dtype=np.float16)
        head_a[p, j] = (h_loc - tile_id * TS).astype(np.float16)

        oh = np.zeros((P, ncol), dtype=np.float16)
        q_r = RP * (j & 3) + etype[e_idx]
        col = (j >> 2) * P + p
        oh[q_r, col] = aug[e_idx]

        in_maps.append(
            {
                "all_emb": emb,
                "idx16": idx16,
                "head_local": head_a,
                "oh": oh,
                "w4": w4,
                "iota": iota,
            }
        )
    return capw, in_maps


def kernel(all_emb, edge_index, edge_type, weight, aug_edge_weight):
    capw, in_maps = _prep(all_emb, edge_index, edge_type, weight,
                          aug_edge_weight)
    if capw not in _NC_CACHE:
        _NC_CACHE[capw] = _build(capw)
    nc = _NC_CACHE[capw]

    trace = bool(int(os.environ.get("KERNEL_TRACE", "0")))
    res = run_bass_kernel_spmd(
        nc,
        in_maps,
        core_ids=list(range(N_CORES)),
        trace=trace,
    )
    kernel._last_results = res
    out = np.concatenate(
        [res.results[c_i]["out"][:NPC] for c_i in range(N_CORES)], axis=0
    )
    return out
